# revision 6
# baseline (speedup 1.0000x reference)
"""Causal self-attention (GQA + RoPE) Trainium2 Bass kernel, v2.

Sharding: 8 cores = 2 (batch) x 4 (kv-head groups). Each core computes the
full attention for one batch element and one kv head (with its 4 query
heads), producing a partial output projection (row-split Wproj); the host
sums the 4 kv-group partials per batch element.

v2 changes vs baseline:
  - x is transposed on the host (free): no PE transposes / DMA-xbar loads.
  - softmax denominator via DVE-accumulated exp tiles + 2 small matmuls
    per (chunk, head) instead of 160 ones-matmuls (-34us tensor).
  - score tiles in f16 PSUM (1 bank/pair) to fit an 8-bank interleave.
  - output projection folded into the attention chunk loop (deferred
    finalize software pipeline); y stored f16, host sums in f32.

Self-contained: hardcodes B=2, T=2048, E=2048, H=16, HKV=4, D=128.
"""

import sys

for _p in ("/opt/trn_rl_repo", "/root/.axon_site/_ro/trn_rl_repo"):
    if _p not in sys.path:
        sys.path.append(_p)

import math
from contextlib import ExitStack

import numpy as np

import concourse.bacc as bacc
import concourse.tile as tile
import concourse.mybir as mybir
from concourse.bass_utils import run_bass_kernel_spmd

P = 128          # partitions
T = 2048         # sequence length
E = 2048         # embed dim
D = 128          # head dim
GH = 4           # query heads per core (= per kv head)
CH = 512         # t-chunk width (PSUM bank = 512 f32)
NCH = T // CH    # 4 t-chunks
NE = E // P      # 16 contraction chunks over E
NK = T // P      # 16 key tiles

F32 = mybir.dt.float32
F16 = mybir.dt.float16
EXPF = mybir.ActivationFunctionType.Exp


def _emit(nc):
    xt = nc.dram_tensor("xt", [E, T], F16, kind="ExternalInput")
    wq = nc.dram_tensor("wq", [E, GH * D], F16, kind="ExternalInput")
    wk = nc.dram_tensor("wk", [E, D], F16, kind="ExternalInput")
    wv = nc.dram_tensor("wv", [E, D], F16, kind="ExternalInput")
    wp = nc.dram_tensor("wp", [GH * D, E], F16, kind="ExternalInput")
    cos = nc.dram_tensor("cos", [D, T], F16, kind="ExternalInput")
    sn = nc.dram_tensor("sn", [D, T], F16, kind="ExternalInput")
    maskp = nc.dram_tensor("maskp", [P, 2, 2 * CH], F16, kind="ExternalInput")
    ident = nc.dram_tensor("ident", [P, P], F16, kind="ExternalInput")
    y = nc.dram_tensor("y", [T, E], F16, kind="ExternalOutput")

    with tile.TileContext(nc) as tc, ExitStack() as ctx:
        # ---- persistent pools (live across phases) ----
        pool_cst = ctx.enter_context(tc.tile_pool(name="cst", bufs=1))
        pool_qfin = ctx.enter_context(tc.tile_pool(name="qfin", bufs=GH))
        pool_kfin = ctx.enter_context(tc.tile_pool(name="kfin", bufs=1))
        pool_vfin = ctx.enter_context(tc.tile_pool(name="vfin", bufs=1))
        pool_outf = ctx.enter_context(tc.tile_pool(name="outf", bufs=GH))
        pool_wp_ = ctx.enter_context(tc.tile_pool(name="wpp", bufs=1))

        ident16 = pool_cst.tile([P, P], F16)
        ones16 = pool_cst.tile([P, P], F16)
        nc.gpsimd.memset(ones16[:], 1.0)
        mask_sb = pool_cst.tile([P, 2, 2 * CH], F16)

        qfin = [pool_qfin.tile([P, T], F16, tag="qfin", name=f"qfin{h}")
                for h in range(GH)]
        kfin = pool_kfin.tile([P, T], F16)
        vfin = pool_vfin.tile([P, NK, P], F16)
        outf = [pool_outf.tile([P, T], F16, tag="outf", name=f"outf{h}")
                for h in range(GH)]
        wp_r = pool_wp_.tile([P, GH, E], F16)

        def rope_combine(dst_slice, psrc, cos_sl, sn_sl, pool):
            # dst = psrc * cos + rotate_half(psrc) * sn (sn rows 0:64 negated)
            raw = pool.tile([P, CH], F16, tag="rp_raw")
            nc.scalar.copy(raw[:], psrc[:])
            sw = pool.tile([P, CH], F16, tag="rp_sw")
            nc.vector.tensor_copy(sw[0:64, :], raw[64:128, :])
            nc.vector.tensor_copy(sw[64:128, :], raw[0:64, :])
            m1 = pool.tile([P, CH], F16, tag="rp_m1")
            nc.vector.tensor_mul(m1[:], raw[:], cos_sl)
            nc.vector.tensor_mul(sw[:], sw[:], sn_sl)
            nc.vector.tensor_add(dst_slice, m1[:], sw[:])

        # ================= Phase B: projections + RoPE =================
        with (
            tc.tile_pool(name="xtp", bufs=1) as pool_xt,
            tc.tile_pool(name="wqp", bufs=1) as pool_wq,
            tc.tile_pool(name="wkv", bufs=1) as pool_wkv,
            tc.tile_pool(name="tab", bufs=1) as pool_tab,
            tc.tile_pool(name="rw", bufs=3) as pool_rw,
            tc.tile_pool(name="vts", bufs=1) as pool_vt,
            tc.tile_pool(name="pspj", bufs=1, space="PSUM") as ps_pj,
            tc.tile_pool(name="pstr", bufs=1, space="PSUM") as ps_tr,
        ):
            vt_sb = pool_vt.tile([P, T], F16)
            xt_sb = pool_xt.tile([P, NE, T], F16)

            # weights first (small, needed first), then x tiles in e-order
            # DMA order matches first-consumption order: c=0 k/v pass
            # needs wk/wv + xt cols 0:512; then wq for the q pass; then
            # the remaining xt column-quarters chunk by chunk.
            wk_r = pool_wkv.tile([P, NE, D], F16, tag="wk")
            nc.sync.dma_start(
                wk_r[:], wk.ap().rearrange("(n p) m -> p n m", p=P))
            wv_r = pool_wkv.tile([P, NE, D], F16, tag="wv")
            nc.sync.dma_start(
                wv_r[:], wv.ap().rearrange("(n p) m -> p n m", p=P))
            for e in range(NE):
                nc.sync.dma_start(xt_sb[:, e, 0:CH],
                                  xt.ap()[e * P:(e + 1) * P, 0:CH])
            wq_r = pool_wq.tile([P, NE, GH * D], F16)
            nc.sync.dma_start(
                wq_r[:], wq.ap().rearrange("(n p) m -> p n m", p=P))
            for e in range(NE):
                nc.sync.dma_start(xt_sb[:, e, CH:2 * CH],
                                  xt.ap()[e * P:(e + 1) * P, CH:2 * CH])
            cos_sb = pool_tab.tile([P, T], F16, tag="cos")
            nc.sync.dma_start(cos_sb[:], cos.ap()[:])
            sn_sb = pool_tab.tile([P, T], F16, tag="sn")
            nc.sync.dma_start(sn_sb[:], sn.ap()[:])
            for e in range(NE):
                nc.sync.dma_start(xt_sb[:, e, 2 * CH:3 * CH],
                                  xt.ap()[e * P:(e + 1) * P, 2 * CH:3 * CH])
            nc.sync.dma_start(mask_sb[:], maskp.ap()[:])
            nc.sync.dma_start(ident16[:], ident.ap()[:])
            for e in range(NE):
                nc.sync.dma_start(xt_sb[:, e, 3 * CH:4 * CH],
                                  xt.ap()[e * P:(e + 1) * P, 3 * CH:4 * CH])

            for c in range(NCH):
                cs = slice(c * CH, (c + 1) * CH)
                pk = ps_pj.tile([P, CH], F32, tag="pk", name=f"pk{c}")
                pv = ps_pj.tile([P, CH], F32, tag="pv", name=f"pv{c}")
                pqs = [ps_pj.tile([P, CH], F32, tag=f"pq{h}",
                                  name=f"pq{c}_{h}") for h in range(GH)]
                if c == 0:
                    # k/v first so the first MMs only need wk/wv + xt tiles
                    for e in range(NE):
                        st = (e == 0)
                        sp = (e == NE - 1)
                        nc.tensor.matmul(pk[:], wk_r[:, e, :],
                                         xt_sb[:, e, cs], start=st, stop=sp)
                        nc.tensor.matmul(pv[:], wv_r[:, e, :],
                                         xt_sb[:, e, cs], start=st, stop=sp)
                    for e in range(NE):
                        for h in range(GH):
                            nc.tensor.matmul(
                                pqs[h][:], wq_r[:, e, h * D:(h + 1) * D],
                                xt_sb[:, e, cs],
                                start=(e == 0), stop=(e == NE - 1))
                else:
                    for e in range(NE):
                        st = (e == 0)
                        sp = (e == NE - 1)
                        nc.tensor.matmul(pk[:], wk_r[:, e, :],
                                         xt_sb[:, e, cs], start=st, stop=sp)
                        nc.tensor.matmul(pv[:], wv_r[:, e, :],
                                         xt_sb[:, e, cs], start=st, stop=sp)
                        for h in range(GH):
                            nc.tensor.matmul(
                                pqs[h][:], wq_r[:, e, h * D:(h + 1) * D],
                                xt_sb[:, e, cs], start=st, stop=sp)
                rope_combine(kfin[:, cs], pk, cos_sb[:, cs], sn_sb[:, cs],
                             pool_rw)
                nc.scalar.copy(vt_sb[:, cs], pv[:])
                # v natural layout via PE transposes
                vtb = ps_tr.tile([P, 4 * P], F16, tag="tr", name=f"vtb{c}")
                for j in range(4):
                    kt = c * 4 + j
                    nc.tensor.matmul(
                        vtb[:, j * P:(j + 1) * P],
                        vt_sb[:, kt * P:(kt + 1) * P],
                        ident16[:],
                        is_transpose=True,
                        start=(j == 0), stop=(j == 3))
                nc.vector.tensor_copy(vfin[:, c * 4:(c + 1) * 4, :], vtb[:])
                for h in range(GH):
                    rope_combine(qfin[h][:, cs], pqs[h],
                                 cos_sb[:, cs], sn_sb[:, cs], pool_rw)

        # wp: needed only by the folded output projection
        for j in range(GH):
            nc.sync.dma_start(wp_r[:, j, :], wp.ap()[j * P:(j + 1) * P, :])

        # ============ Phase C+D: attention + folded projection ============
        with (
            tc.tile_pool(name="expb", bufs=4) as pool_exp,
            tc.tile_pool(name="accb", bufs=2) as pool_acc,
            tc.tile_pool(name="accg", bufs=2) as pool_accg,
            tc.tile_pool(name="accm", bufs=2) as pool_accm,
            tc.tile_pool(name="recb", bufs=2) as pool_rec,
            tc.tile_pool(name="ystg", bufs=3) as pool_y,
            tc.tile_pool(name="scps", bufs=2, space="PSUM") as ps_sc,
            tc.tile_pool(name="avps", bufs=2, space="PSUM") as ps_av,
            tc.tile_pool(name="smps", bufs=1, space="PSUM") as ps_sm,
            tc.tile_pool(name="pyps", bufs=1, space="PSUM") as ps_py,
        ):
            dq = []  # pending output-projection group closures

            def pop_d():
                if dq:
                    dq.pop(0)()

            def attn(c, h):
                """Scores, exp, mask, DVE-accumulate, AV for one (c, h)."""
                nk = 4 * c + 4
                npair = nk // 2
                cs = slice(c * CH, (c + 1) * CH)
                av = ps_av.tile([P, CH], F32, tag="av", name=f"av{c}_{h}")
                acc_d = pool_acc.tile([P, 2 * CH], F16, tag="acc",
                                      name=f"accd{c}_{h}")
                acc_g = (pool_accg.tile([P, 2 * CH], F16, tag="accg",
                                        name=f"accg{c}_{h}")
                         if npair >= 4 else None)
                exps = {}
                heads = {}
                for kp in range(npair + 1):
                    if kp < npair:
                        sc = ps_sc.tile([P, 2 * CH], F32, tag="sc",
                                        name=f"sc{c}_{h}_{kp}")
                        for half in (0, 1):
                            k = 2 * kp + half
                            nc.tensor.matmul(
                                sc[:, half * CH:(half + 1) * CH],
                                kfin[:, k * P:(k + 1) * P],
                                qfin[h][:, cs],
                                start=True, stop=True)
                        ex = pool_exp.tile([P, 2 * CH], F16, tag="ex",
                                           name=f"ex{c}_{h}_{kp}")
                        nc.scalar.activation(ex[:], sc[:], EXPF)
                        if kp >= npair - 2:
                            # diagonal pair: zero above-diagonal entries
                            mp = kp - (npair - 2)
                            nc.vector.tensor_mul(ex[:], ex[:],
                                                 mask_sb[:, mp, :])
                        # accumulate exp for the softmax denominator:
                        # even pairs chain on DVE, odd pairs on gpsimd
                        if npair < 4:
                            if kp == 1:
                                nc.vector.tensor_add(acc_d[:], exps[0][:],
                                                     ex[:])
                        elif kp < 2:
                            heads[kp % 2] = ex
                        elif kp < 4:
                            eng = nc.vector if kp % 2 == 0 else nc.gpsimd
                            dst = acc_d if kp % 2 == 0 else acc_g
                            eng.tensor_add(dst[:], heads.pop(kp % 2)[:],
                                           ex[:])
                        else:
                            eng = nc.vector if kp % 2 == 0 else nc.gpsimd
                            dst = acc_d if kp % 2 == 0 else acc_g
                            eng.tensor_add(dst[:], dst[:], ex[:])
                    if kp >= 1:
                        exd = exps.pop(kp - 1)
                        for half in (0, 1):
                            k = 2 * (kp - 1) + half
                            nc.tensor.matmul(
                                av[:], vfin[:, k, :],
                                exd[:, half * CH:(half + 1) * CH],
                                start=(k == 0), stop=(k == nk - 1))
                        # rate-match PE vs ACT: slot one projection group
                        # (4 matmuls) into the pair pipeline
                        pop_d()
                    if kp < npair:
                        exps[kp] = ex
                return av, acc_d, acc_g

            def finalize(c, h, av, acc_d, acc_g):
                """Denominator broadcast-reduce, reciprocal, normalize."""
                cs = slice(c * CH, (c + 1) * CH)
                if acc_g is not None:
                    acc = pool_accm.tile([P, 2 * CH], F16, tag="accm",
                                         name=f"accm{c}_{h}")
                    nc.vector.tensor_add(acc[:], acc_d[:], acc_g[:])
                else:
                    acc = acc_d
                sm = ps_sm.tile([P, CH], F32, tag="sm", name=f"sm{c}_{h}")
                nc.tensor.matmul(sm[:], ones16[:], acc[:, 0:CH],
                                 start=True, stop=False)
                nc.tensor.matmul(sm[:], ones16[:], acc[:, CH:2 * CH],
                                 start=False, stop=True)
                rec = pool_rec.tile([P, CH], F32, tag="rec",
                                    name=f"rec{c}_{h}")
                nc.vector.reciprocal_approx_fast(rec[:], sm[:])
                nc.vector.tensor_mul(outf[h][:, cs], av[:], rec[:])

            def queue_d(c):
                """Queue output projection + store groups for t-chunk c."""
                def mk(t, eo):
                    def emit():
                        py = ps_py.tile([P, CH], F32, tag="py",
                                        name=f"py{c}_{t}_{eo}")
                        for j in range(GH):
                            nc.tensor.matmul(
                                py[:],
                                outf[j][:, t * P:(t + 1) * P],
                                wp_r[:, j, eo * CH:(eo + 1) * CH],
                                start=(j == 0), stop=(j == GH - 1))
                        ys = pool_y.tile([P, CH], F16, tag="ys")
                        nc.vector.tensor_copy(ys[:], py[:])
                        nc.sync.dma_start(
                            y.ap()[t * P:(t + 1) * P, eo * CH:(eo + 1) * CH],
                            ys[:])
                    return emit
                for tt in range(4):
                    for eo in range(4):
                        dq.append(mk(4 * c + tt, eo))

            pending = None
            for c in range(NCH):
                for h in range(GH):
                    cur = attn(c, h)
                    if pending is not None:
                        finalize(*pending)
                        if pending[1] == GH - 1:
                            queue_d(pending[0])
                    pending = (c, h) + cur
            finalize(*pending)
            queue_d(NCH - 1)
            while dq:
                pop_d()

    return nc


_NC = None


def build_nc():
    global _NC
    if _NC is None:
        nc = bacc.Bacc("TRN2", target_bir_lowering=False, debug=False)
        _emit(nc)
        nc.compile()
        _NC = nc
    return _NC


def host_tables(pos):
    """RoPE tables, exactly mirroring the reference construction."""
    half = D // 2
    inv_freq = (1.0 / np.power(10000.0, np.arange(0, D, 2, dtype=np.float32) / D))
    t = np.arange(pos, pos + T, dtype=np.float32)
    freqs = t[:, None] * inv_freq[None, :]          # [T, half]
    freqs = np.repeat(freqs, 2, axis=-1)            # [T, D]
    cos = np.cos(freqs).astype(np.float32).T.copy() # [D, T]
    sin = np.sin(freqs).astype(np.float32).T.copy() # [D, T]
    sn = sin.copy()
    sn[:half] = -sn[:half]
    return (np.ascontiguousarray(cos).astype(np.float16),
            np.ascontiguousarray(sn).astype(np.float16))


def host_masks():
    # pairs of diagonal masks: pair 0 = offsets (0, 1), pair 1 = (2, 3);
    # mask(m)[p, q] = 1 where key p + 128*m <= query q (q in [0, 512))
    kk = np.arange(P)[:, None]
    qq = np.arange(CH)[None, :]
    masks = [(kk + 128 * m <= qq) for m in range(4)]
    pair0 = np.concatenate([masks[0], masks[1]], axis=1)  # [P, 1024]
    pair1 = np.concatenate([masks[2], masks[3]], axis=1)
    m = np.stack([pair0, pair1], axis=1)                  # [P, 2, 1024]
    return m.astype(np.float16)


def make_in_maps(x, Wq, Wk, Wv, Wproj, pos):
    x = np.asarray(x, dtype=np.float32)
    Wq = np.asarray(Wq, dtype=np.float32)
    Wk = np.asarray(Wk, dtype=np.float32)
    Wv = np.asarray(Wv, dtype=np.float32)
    Wproj = np.asarray(Wproj, dtype=np.float32)
    scale = np.float32(1.0 / math.sqrt(D))
    cos, sn = host_tables(int(pos))
    mask = host_masks()
    ident = np.eye(P, dtype=np.float16)
    in_maps = []
    for c in range(8):
        b, g = divmod(c, 4)
        xtb = np.ascontiguousarray(x[b].T).astype(np.float16)
        in_maps.append({
            "xt": xtb,
            "wq": np.ascontiguousarray(
                Wq[:, g * GH * D:(g + 1) * GH * D] * scale).astype(np.float16),
            "wk": np.ascontiguousarray(Wk[:, g * D:(g + 1) * D]).astype(np.float16),
            "wv": np.ascontiguousarray(Wv[:, g * D:(g + 1) * D]).astype(np.float16),
            "wp": np.ascontiguousarray(
                Wproj[g * GH * D:(g + 1) * GH * D, :]).astype(np.float16),
            "cos": cos,
            "sn": sn,
            "maskp": mask,
            "ident": ident,
        })
    return in_maps


def kernel_with_results(x, Wq, Wk, Wv, Wproj, pos, trace=False):
    nc = build_nc()
    in_maps = make_in_maps(x, Wq, Wk, Wv, Wproj, pos)
    res = run_bass_kernel_spmd(nc, in_maps, list(range(8)), trace=trace)
    B = 2
    y = np.zeros((B, T, E), dtype=np.float32)
    for c in range(8):
        b = c // 4
        y[b] += res.results[c]["y"].astype(np.float32)
    return y, res


def kernel(x, Wq, Wk, Wv, Wproj, pos):
    y, _ = kernel_with_results(x, Wq, Wk, Wv, Wproj, pos)
    return y


# revision 8
# speedup vs baseline: 1.0841x; 1.0841x over previous
"""Causal self-attention (GQA + RoPE) Trainium2 Bass kernel, v2.

Sharding: 8 cores = 2 (batch) x 4 (kv-head groups). Each core computes the
full attention for one batch element and one kv head (with its 4 query
heads), producing a partial output projection (row-split Wproj); the host
sums the 4 kv-group partials per batch element.

v2 changes vs baseline:
  - x is transposed on the host (free): no PE transposes / DMA-xbar loads.
  - softmax denominator via DVE-accumulated exp tiles + 2 small matmuls
    per (chunk, head) instead of 160 ones-matmuls (-34us tensor).
  - score tiles in f16 PSUM (1 bank/pair) to fit an 8-bank interleave.
  - output projection folded into the attention chunk loop (deferred
    finalize software pipeline); y stored f16, host sums in f32.

Self-contained: hardcodes B=2, T=2048, E=2048, H=16, HKV=4, D=128.
"""

import sys

for _p in ("/opt/trn_rl_repo", "/root/.axon_site/_ro/trn_rl_repo"):
    if _p not in sys.path:
        sys.path.append(_p)

import math
from contextlib import ExitStack

import numpy as np

import concourse.bacc as bacc
import concourse.tile as tile
import concourse.mybir as mybir
from concourse.bass_utils import run_bass_kernel_spmd

P = 128          # partitions
T = 2048         # sequence length
E = 2048         # embed dim
D = 128          # head dim
GH = 4           # query heads per core (= per kv head)
CH = 512         # t-chunk width (PSUM bank = 512 f32)
NCH = T // CH    # 4 t-chunks
NE = E // P      # 16 contraction chunks over E
NK = T // P      # 16 key tiles

F32 = mybir.dt.float32
F16 = mybir.dt.float16
EXPF = mybir.ActivationFunctionType.Exp


def _emit(nc):
    xt = nc.dram_tensor("xt", [NCH, E, CH], F16, kind="ExternalInput")
    wq = nc.dram_tensor("wq", [P, NE, GH * D], F16, kind="ExternalInput")
    wk = nc.dram_tensor("wk", [P, NE, D], F16, kind="ExternalInput")
    wv = nc.dram_tensor("wv", [P, NE, D], F16, kind="ExternalInput")
    wp = nc.dram_tensor("wp", [GH * D, E], F16, kind="ExternalInput")
    cos = nc.dram_tensor("cos", [D, T], F16, kind="ExternalInput")
    sn = nc.dram_tensor("sn", [D, T], F16, kind="ExternalInput")
    maskp = nc.dram_tensor("maskp", [P, 2, 2 * CH], F16, kind="ExternalInput")
    ident = nc.dram_tensor("ident", [P, P], F16, kind="ExternalInput")
    y = nc.dram_tensor("y", [T, E], F16, kind="ExternalOutput")

    with tile.TileContext(nc) as tc, ExitStack() as ctx:
        # ---- persistent pools (live across phases) ----
        pool_cst = ctx.enter_context(tc.tile_pool(name="cst", bufs=1))
        pool_qfin = ctx.enter_context(tc.tile_pool(name="qfin", bufs=GH))
        pool_kfin = ctx.enter_context(tc.tile_pool(name="kfin", bufs=1))
        pool_vfin = ctx.enter_context(tc.tile_pool(name="vfin", bufs=1))
        pool_outf = ctx.enter_context(tc.tile_pool(name="outf", bufs=GH))
        pool_wp_ = ctx.enter_context(tc.tile_pool(name="wpp", bufs=1))

        ident16 = pool_cst.tile([P, P], F16)
        ones16 = pool_cst.tile([P, P], F16)
        nc.gpsimd.memset(ones16[:], 1.0)
        mask_sb = pool_cst.tile([P, 2, 2 * CH], F16)

        qfin = [pool_qfin.tile([P, T], F16, tag="qfin", name=f"qfin{h}")
                for h in range(GH)]
        kfin = pool_kfin.tile([P, T], F16)
        vfin = pool_vfin.tile([P, NK, P], F16)
        outf = [pool_outf.tile([P, T], F16, tag="outf", name=f"outf{h}")
                for h in range(GH)]
        wp_r = pool_wp_.tile([P, GH, E], F16)

        def rope_combine(dst_slice, psrc, cos_sl, sn_sl, pool):
            # dst = psrc * cos + rotate_half(psrc) * sn (sn rows 0:64 negated)
            raw = pool.tile([P, CH], F16, tag="rp_raw")
            nc.scalar.copy(raw[:], psrc[:])
            sw = pool.tile([P, CH], F16, tag="rp_sw")
            nc.vector.tensor_copy(sw[0:64, :], raw[64:128, :])
            nc.vector.tensor_copy(sw[64:128, :], raw[0:64, :])
            m1 = pool.tile([P, CH], F16, tag="rp_m1")
            nc.vector.tensor_mul(m1[:], raw[:], cos_sl)
            nc.vector.tensor_mul(sw[:], sw[:], sn_sl)
            nc.vector.tensor_add(dst_slice, m1[:], sw[:])

        # ================= Phase B: projections + RoPE =================
        with (
            tc.tile_pool(name="xtp", bufs=1) as pool_xt,
            tc.tile_pool(name="wqp", bufs=1) as pool_wq,
            tc.tile_pool(name="wkv", bufs=1) as pool_wkv,
            tc.tile_pool(name="tab", bufs=1) as pool_tab,
            tc.tile_pool(name="rw", bufs=3) as pool_rw,
            tc.tile_pool(name="vts", bufs=1) as pool_vt,
            tc.tile_pool(name="pspj", bufs=1, space="PSUM") as ps_pj,
            tc.tile_pool(name="pstr", bufs=1, space="PSUM") as ps_tr,
        ):
            vt_sb = pool_vt.tile([P, T], F16)
            xt_sb = pool_xt.tile([P, NE, T], F16)

            # weights first (small, needed first), then x tiles in e-order
            # DMA order matches first-consumption order: c=0 k/v pass
            # needs wk/wv + xt cols 0:512; then wq for the q pass; then
            # the remaining xt column-quarters chunk by chunk.
            wk_r = pool_wkv.tile([P, NE, D], F16, tag="wk")
            nc.sync.dma_start(wk_r[:], wk.ap()[:])
            wv_r = pool_wkv.tile([P, NE, D], F16, tag="wv")
            nc.sync.dma_start(wv_r[:], wv.ap()[:])
            for e in range(NE):
                nc.sync.dma_start(xt_sb[:, e, 0:CH],
                                  xt.ap()[0, e * P:(e + 1) * P, :])
            wq_r = pool_wq.tile([P, NE, GH * D], F16)
            nc.sync.dma_start(wq_r[:], wq.ap()[:])
            for e in range(NE):
                nc.sync.dma_start(xt_sb[:, e, CH:2 * CH],
                                  xt.ap()[1, e * P:(e + 1) * P, :])
            cos_sb = pool_tab.tile([P, T], F16, tag="cos")
            nc.sync.dma_start(cos_sb[:], cos.ap()[:])
            sn_sb = pool_tab.tile([P, T], F16, tag="sn")
            nc.sync.dma_start(sn_sb[:], sn.ap()[:])
            for e in range(NE):
                nc.sync.dma_start(xt_sb[:, e, 2 * CH:3 * CH],
                                  xt.ap()[2, e * P:(e + 1) * P, :])
            nc.sync.dma_start(mask_sb[:], maskp.ap()[:])
            nc.sync.dma_start(ident16[:], ident.ap()[:])
            for e in range(NE):
                nc.sync.dma_start(xt_sb[:, e, 3 * CH:4 * CH],
                                  xt.ap()[3, e * P:(e + 1) * P, :])

            for c in range(NCH):
                cs = slice(c * CH, (c + 1) * CH)
                pk = ps_pj.tile([P, CH], F32, tag="pk", name=f"pk{c}")
                pv = ps_pj.tile([P, CH], F32, tag="pv", name=f"pv{c}")
                pqs = [ps_pj.tile([P, CH], F32, tag=f"pq{h}",
                                  name=f"pq{c}_{h}") for h in range(GH)]
                if c == 0:
                    # k/v first so the first MMs only need wk/wv + xt tiles
                    for e in range(NE):
                        st = (e == 0)
                        sp = (e == NE - 1)
                        nc.tensor.matmul(pk[:], wk_r[:, e, :],
                                         xt_sb[:, e, cs], start=st, stop=sp)
                        nc.tensor.matmul(pv[:], wv_r[:, e, :],
                                         xt_sb[:, e, cs], start=st, stop=sp)
                    for e in range(NE):
                        for h in range(GH):
                            nc.tensor.matmul(
                                pqs[h][:], wq_r[:, e, h * D:(h + 1) * D],
                                xt_sb[:, e, cs],
                                start=(e == 0), stop=(e == NE - 1))
                else:
                    for e in range(NE):
                        st = (e == 0)
                        sp = (e == NE - 1)
                        nc.tensor.matmul(pk[:], wk_r[:, e, :],
                                         xt_sb[:, e, cs], start=st, stop=sp)
                        nc.tensor.matmul(pv[:], wv_r[:, e, :],
                                         xt_sb[:, e, cs], start=st, stop=sp)
                        for h in range(GH):
                            nc.tensor.matmul(
                                pqs[h][:], wq_r[:, e, h * D:(h + 1) * D],
                                xt_sb[:, e, cs], start=st, stop=sp)
                rope_combine(kfin[:, cs], pk, cos_sb[:, cs], sn_sb[:, cs],
                             pool_rw)
                nc.scalar.copy(vt_sb[:, cs], pv[:])
                # v natural layout via PE transposes
                vtb = ps_tr.tile([P, 4 * P], F16, tag="tr", name=f"vtb{c}")
                for j in range(4):
                    kt = c * 4 + j
                    nc.tensor.matmul(
                        vtb[:, j * P:(j + 1) * P],
                        vt_sb[:, kt * P:(kt + 1) * P],
                        ident16[:],
                        is_transpose=True,
                        start=(j == 0), stop=(j == 3))
                nc.vector.tensor_copy(vfin[:, c * 4:(c + 1) * 4, :], vtb[:])
                for h in range(GH):
                    rope_combine(qfin[h][:, cs], pqs[h],
                                 cos_sb[:, cs], sn_sb[:, cs], pool_rw)

        # wp: needed only by the folded output projection
        for j in range(GH):
            nc.sync.dma_start(wp_r[:, j, :], wp.ap()[j * P:(j + 1) * P, :])

        # ============ Phase C+D: attention + folded projection ============
        with (
            tc.tile_pool(name="expb", bufs=4) as pool_exp,
            tc.tile_pool(name="accb", bufs=2) as pool_acc,
            tc.tile_pool(name="accg", bufs=2) as pool_accg,
            tc.tile_pool(name="accm", bufs=2) as pool_accm,
            tc.tile_pool(name="recb", bufs=2) as pool_rec,
            tc.tile_pool(name="ystg", bufs=3) as pool_y,
            tc.tile_pool(name="scps", bufs=2, space="PSUM") as ps_sc,
            tc.tile_pool(name="avps", bufs=2, space="PSUM") as ps_av,
            tc.tile_pool(name="pyps", bufs=2, space="PSUM") as ps_py,
        ):
            dq = []  # pending output-projection group closures

            def pop_d():
                if dq:
                    dq.pop(0)()

            def attn(c, h):
                """Scores, exp, mask, DVE-accumulate, AV for one (c, h)."""
                nk = 4 * c + 4
                npair = nk // 2
                cs = slice(c * CH, (c + 1) * CH)
                av = ps_av.tile([P, CH], F32, tag="av", name=f"av{c}_{h}")
                acc_d = pool_acc.tile([P, 2 * CH], F16, tag="acc",
                                      name=f"accd{c}_{h}")
                acc_g = (pool_accg.tile([P, 2 * CH], F16, tag="accg",
                                        name=f"accg{c}_{h}")
                         if npair >= 4 else None)
                exps = {}
                heads = {}
                for kp in range(npair + 1):
                    if kp < npair:
                        sc = ps_sc.tile([P, 2 * CH], F32, tag="sc",
                                        name=f"sc{c}_{h}_{kp}")
                        for half in (0, 1):
                            k = 2 * kp + half
                            nc.tensor.matmul(
                                sc[:, half * CH:(half + 1) * CH],
                                kfin[:, k * P:(k + 1) * P],
                                qfin[h][:, cs],
                                start=True, stop=True)
                        ex = pool_exp.tile([P, 2 * CH], F16, tag="ex",
                                           name=f"ex{c}_{h}_{kp}")
                        nc.scalar.activation(ex[:], sc[:], EXPF)
                        if kp >= npair - 2:
                            # diagonal pair: zero above-diagonal entries
                            mp = kp - (npair - 2)
                            nc.vector.tensor_mul(ex[:], ex[:],
                                                 mask_sb[:, mp, :])
                        # accumulate exp for the softmax denominator:
                        # even pairs chain on DVE, odd pairs on gpsimd
                        if npair < 4:
                            if kp == 1:
                                nc.vector.tensor_add(acc_d[:], exps[0][:],
                                                     ex[:])
                        elif kp < 2:
                            heads[kp % 2] = ex
                        elif kp < 4:
                            eng = nc.vector if kp % 2 == 0 else nc.gpsimd
                            dst = acc_d if kp % 2 == 0 else acc_g
                            eng.tensor_add(dst[:], heads.pop(kp % 2)[:],
                                           ex[:])
                        else:
                            eng = nc.vector if kp % 2 == 0 else nc.gpsimd
                            dst = acc_d if kp % 2 == 0 else acc_g
                            eng.tensor_add(dst[:], dst[:], ex[:])
                    if kp >= 1:
                        exd = exps.pop(kp - 1)
                        for half in (0, 1):
                            k = 2 * (kp - 1) + half
                            nc.tensor.matmul(
                                av[:], vfin[:, k, :],
                                exd[:, half * CH:(half + 1) * CH],
                                start=(k == 0), stop=(k == nk - 1))
                        # rate-match PE vs ACT: slot one projection group
                        # (4 matmuls) into the pair pipeline
                        pop_d()
                    if kp < npair:
                        exps[kp] = ex
                return av, acc_d, acc_g

            def finalize(c, h, av, acc_d, acc_g):
                """Denominator broadcast-reduce, reciprocal, normalize."""
                cs = slice(c * CH, (c + 1) * CH)
                if acc_g is not None:
                    acc = pool_accm.tile([P, 2 * CH], F16, tag="accm",
                                         name=f"accm{c}_{h}")
                    nc.vector.tensor_add(acc[:], acc_d[:], acc_g[:])
                else:
                    acc = acc_d
                sm = ps_py.tile([P, CH], F32, tag="py", name=f"sm{c}_{h}")
                nc.tensor.matmul(sm[:], ones16[:], acc[:, 0:CH],
                                 start=True, stop=False)
                nc.tensor.matmul(sm[:], ones16[:], acc[:, CH:2 * CH],
                                 start=False, stop=True)
                rec = pool_rec.tile([P, CH], F32, tag="rec",
                                    name=f"rec{c}_{h}")
                nc.vector.reciprocal_approx_fast(rec[:], sm[:])
                nc.vector.tensor_mul(outf[h][:, cs], av[:], rec[:])

            def queue_d(c):
                """Queue output projection + store groups for t-chunk c."""
                def mk(t, eo):
                    def emit():
                        py = ps_py.tile([P, CH], F32, tag="py",
                                        name=f"py{c}_{t}_{eo}")
                        for j in range(GH):
                            nc.tensor.matmul(
                                py[:],
                                outf[j][:, t * P:(t + 1) * P],
                                wp_r[:, j, eo * CH:(eo + 1) * CH],
                                start=(j == 0), stop=(j == GH - 1))
                        ys = pool_y.tile([P, CH], F16, tag="ys")
                        nc.vector.tensor_copy(ys[:], py[:])
                        nc.sync.dma_start(
                            y.ap()[t * P:(t + 1) * P, eo * CH:(eo + 1) * CH],
                            ys[:])
                    return emit
                for tt in range(4):
                    for eo in range(4):
                        dq.append(mk(4 * c + tt, eo))

            pending = None
            for c in range(NCH):
                for h in range(GH):
                    cur = attn(c, h)
                    if pending is not None:
                        finalize(*pending)
                        if pending[1] == GH - 1:
                            queue_d(pending[0])
                    pending = (c, h) + cur
            finalize(*pending)
            queue_d(NCH - 1)
            while dq:
                pop_d()

    return nc


_NC = None


def build_nc():
    global _NC
    if _NC is None:
        nc = bacc.Bacc("TRN2", target_bir_lowering=False, debug=False)
        _emit(nc)
        nc.compile()
        _NC = nc
    return _NC


def host_tables(pos):
    """RoPE tables, exactly mirroring the reference construction."""
    half = D // 2
    inv_freq = (1.0 / np.power(10000.0, np.arange(0, D, 2, dtype=np.float32) / D))
    t = np.arange(pos, pos + T, dtype=np.float32)
    freqs = t[:, None] * inv_freq[None, :]          # [T, half]
    freqs = np.repeat(freqs, 2, axis=-1)            # [T, D]
    cos = np.cos(freqs).astype(np.float32).T.copy() # [D, T]
    sin = np.sin(freqs).astype(np.float32).T.copy() # [D, T]
    sn = sin.copy()
    sn[:half] = -sn[:half]
    return (np.ascontiguousarray(cos).astype(np.float16),
            np.ascontiguousarray(sn).astype(np.float16))


def host_masks():
    # pairs of diagonal masks: pair 0 = offsets (0, 1), pair 1 = (2, 3);
    # mask(m)[p, q] = 1 where key p + 128*m <= query q (q in [0, 512))
    kk = np.arange(P)[:, None]
    qq = np.arange(CH)[None, :]
    masks = [(kk + 128 * m <= qq) for m in range(4)]
    pair0 = np.concatenate([masks[0], masks[1]], axis=1)  # [P, 1024]
    pair1 = np.concatenate([masks[2], masks[3]], axis=1)
    m = np.stack([pair0, pair1], axis=1)                  # [P, 2, 1024]
    return m.astype(np.float16)


def make_in_maps(x, Wq, Wk, Wv, Wproj, pos):
    x = np.asarray(x, dtype=np.float32)
    Wq = np.asarray(Wq, dtype=np.float32)
    Wk = np.asarray(Wk, dtype=np.float32)
    Wv = np.asarray(Wv, dtype=np.float32)
    Wproj = np.asarray(Wproj, dtype=np.float32)
    scale = np.float32(1.0 / math.sqrt(D))
    cos, sn = host_tables(int(pos))
    mask = host_masks()
    ident = np.eye(P, dtype=np.float16)
    in_maps = []
    for c in range(8):
        b, g = divmod(c, 4)
        xtb = x[b].T.astype(np.float16)              # [E, T]
        xtq = np.ascontiguousarray(
            xtb.reshape(E, NCH, CH).transpose(1, 0, 2))  # [NCH, E, CH]
        wq_h = np.ascontiguousarray(
            (Wq[:, g * GH * D:(g + 1) * GH * D] * scale).astype(np.float16)
            .reshape(NE, P, GH * D).transpose(1, 0, 2))  # [P, NE, 512]
        wk_h = np.ascontiguousarray(
            Wk[:, g * D:(g + 1) * D].astype(np.float16)
            .reshape(NE, P, D).transpose(1, 0, 2))       # [P, NE, D]
        wv_h = np.ascontiguousarray(
            Wv[:, g * D:(g + 1) * D].astype(np.float16)
            .reshape(NE, P, D).transpose(1, 0, 2))
        in_maps.append({
            "xt": xtq,
            "wq": wq_h,
            "wk": wk_h,
            "wv": wv_h,
            "wp": np.ascontiguousarray(
                Wproj[g * GH * D:(g + 1) * GH * D, :]).astype(np.float16),
            "cos": cos,
            "sn": sn,
            "maskp": mask,
            "ident": ident,
        })
    return in_maps


def kernel_with_results(x, Wq, Wk, Wv, Wproj, pos, trace=False):
    nc = build_nc()
    in_maps = make_in_maps(x, Wq, Wk, Wv, Wproj, pos)
    res = run_bass_kernel_spmd(nc, in_maps, list(range(8)), trace=trace)
    B = 2
    y = np.zeros((B, T, E), dtype=np.float32)
    for c in range(8):
        b = c // 4
        y[b] += res.results[c]["y"].astype(np.float32)
    return y, res


def kernel(x, Wq, Wk, Wv, Wproj, pos):
    y, _ = kernel_with_results(x, Wq, Wk, Wv, Wproj, pos)
    return y
